# revision 1
# baseline (speedup 1.0000x reference)
"""7x7 grayscale dilation (flat SE, zero padding) on Trainium2, 8 NeuronCores.

Strategy (pure data parallel, per sharding hint):
  - shard x (32,3,512,512) by batch: 4 batches -> 12 images of 512x512 per core
  - per image: horizontal 7-window max cascade (shifts 1,2,3) along the free
    dim, PE transpose (via identity matmul) to flip W into partitions,
    vertical cascade along the free dim, PE transpose back, store.
  - all maxes on DVE (the only engine supporting TensorTensor in this stack);
    PSUM->SBUF copies on the scalar (ACT) engine; loads/stores on sync (HWDGE).

se is (7,7) ones in this problem: bias = se-1 = 0 and mask = 1, so the op is
exactly a 7x7 sliding max over the zero-padded input.  A numpy fallback
handles any other se faithfully.
"""
import numpy as np

_CACHE = {}

N_CORES = 8
IMGS = 12  # images per core: 4 batches x 3 channels
H = W = 512


def _build_nc(group=1, nslot=6, headsplit=True, tailsplit=True, p_bufs=2, p2_fine=True, p2_bufs=4):
    """group: images per DVE op-group. nslot: rotating buffer slots (of groups)."""
    from contextlib import ExitStack
    from concourse import bacc, tile, mybir
    from concourse.masks import make_identity

    F32 = mybir.dt.float32
    MAX = mybir.AluOpType.max
    G = group
    NG = IMGS // G

    nc = bacc.Bacc("TRN2", target_bir_lowering=False)
    x_in = nc.dram_tensor("x", [IMGS, H, W], F32, kind="ExternalInput")
    y_out = nc.dram_tensor("y", [IMGS, H, W], F32, kind="ExternalOutput")

    with tile.TileContext(nc) as tc:
        with ExitStack() as ctx:
            pool = ctx.enter_context(tc.tile_pool(name="p", bufs=1))
            psum = ctx.enter_context(tc.tile_pool(name="ps", bufs=p_bufs, space="PSUM"))
            psum2 = ctx.enter_context(tc.tile_pool(name="ps2", bufs=p2_bufs, space="PSUM"))

            ident = pool.tile([128, 128], F32)
            make_identity(nc, ident[:])

            FD = 4 * G
            slots = []
            for s in range(nslot):
                b_xt = pool.tile([128, FD, 518], F32, tag=f"xt{s}")
                b_a = pool.tile([128, FD, 517], F32, tag=f"a{s}")
                b_u = pool.tile([128, FD, 517], F32, tag=f"u{s}")
                b_vt = pool.tile([128, FD, 518], F32, tag=f"vt{s}")
                # persistent zero halo columns (never rewritten)
                for t in (b_xt, b_vt):
                    nc.gpsimd.memset(t[:, :, 0:3], 0.0)
                    nc.gpsimd.memset(t[:, :, 515:518], 0.0)
                slots.append((b_xt, b_a, b_u, b_vt))

            def casc(src, b_a, b_u, fsplit=1):
                """7-window max cascade along the last dim.
                src [128,FD,518] (zero halo) -> v in b_a[:, :, 0:512]."""
                step = max(1, FD // fsplit)
                for c0 in range(0, FD, step):
                    c1 = min(c0 + step, FD)
                    nc.vector.tensor_tensor(
                        b_a[:, c0:c1, 0:517], src[:, c0:c1, 0:517],
                        src[:, c0:c1, 1:518], op=MAX)
                    nc.vector.tensor_tensor(
                        b_u[:, c0:c1, 0:515], b_a[:, c0:c1, 0:515],
                        b_a[:, c0:c1, 2:517], op=MAX)
                    nc.vector.tensor_tensor(
                        b_a[:, c0:c1, 0:512], b_u[:, c0:c1, 0:512],
                        b_u[:, c0:c1, 3:515], op=MAX)

            def emit_loads(g, split=False):
                b_xt = slots[g % nslot][0]
                for li in range(G):
                    i = g * G + li
                    src = x_in[i].rearrange("(t p) w -> p t w", p=128, t=4)
                    if split:
                        for T in range(4):
                            eng = nc.sync if T % 2 == 0 else nc.scalar
                            eng.dma_start(
                                out=b_xt[:, 4 * li + T : 4 * li + T + 1, 3:515],
                                in_=src[:, T : T + 1, :],
                            )
                    else:
                        eng = nc.sync if g % 2 == 0 else nc.scalar
                        eng.dma_start(out=b_xt[:, 4 * li : 4 * li + 4, 3:515], in_=src)

            emit_loads(0, split=True)
            for g in range(NG):
                b_xt, b_a, b_u, b_vt = slots[g % nslot]
                first, last = g <= 1, g >= NG - 2

                if g + 1 < NG:
                    emit_loads(g + 1)

                # horizontal cascade; v -> b_a[:, :, 0:512]
                casc(b_xt, b_a, b_u, fsplit=(FD if (headsplit and first) else 1))

                # transpose v -> vT
                for li in range(G):
                    for pair in range(2):  # Wb pairs
                        Pt = psum.tile([128, 1024], F32, tag="P")
                        for wp in range(2):
                            Wb = 2 * pair + wp
                            for T in range(4):
                                nc.tensor.matmul(
                                    Pt[:, 512 * wp + 128 * T : 512 * wp + 128 * T + 128],
                                    b_a[:, 4 * li + T, 128 * Wb : 128 * Wb + 128],
                                    ident[:],
                                    is_transpose=True,
                                )
                        nc.scalar.copy(
                            b_vt[:, 4 * li + 2 * pair : 4 * li + 2 * pair + 2, 3:515],
                            Pt[:].rearrange("p (a b) -> p a b", a=2, b=512),
                        )

                # vertical cascade: a2 -> b_a, u2 -> b_u, z -> b_vt interior
                nc.vector.tensor_tensor(
                    b_a[:, :, 0:517], b_vt[:, :, 0:517], b_vt[:, :, 1:518], op=MAX)
                nc.vector.tensor_tensor(
                    b_u[:, :, 0:515], b_a[:, :, 0:515], b_a[:, :, 2:517], op=MAX)
                zs = (4 if g == NG - 1 else 2) if (tailsplit and last) else 1
                step = 512 // zs
                for c0 in range(0, 512, step):
                    c1 = c0 + step
                    nc.vector.tensor_tensor(
                        b_vt[:, :, 3 + c0 : 3 + c1],
                        b_u[:, :, c0:c1],
                        b_u[:, :, 3 + c0 : 3 + c1],
                        op=MAX)

                # transpose back + store per image
                for li in range(G):
                    i = g * G + li
                    nT = 4 if p2_fine else 2
                    for pair in range(nT):  # T chunks
                        tw = 4 // nT
                        P2 = psum2.tile([128, 512 * tw], F32, tag="P2")
                        for tp in range(tw):
                            T = tw * pair + tp
                            for Wb in range(4):
                                nc.tensor.matmul(
                                    P2[:, 512 * tp + 128 * Wb : 512 * tp + 128 * Wb + 128],
                                    b_vt[:, 4 * li + Wb, 3 + 128 * T : 3 + 128 * T + 128],
                                    ident[:],
                                    is_transpose=True,
                                )
                        nc.scalar.copy(
                            b_xt[:, 4 * li + tw * pair : 4 * li + tw * pair + tw, 3:515],
                            P2[:].rearrange("p (a b) -> p a b", a=tw, b=512),
                        )
                        if tailsplit and last:
                            seng = nc.sync if pair % 2 == 0 else nc.scalar
                            seng.dma_start(
                                out=y_out[i].rearrange(
                                    "(t p) w -> p t w", p=128, t=4
                                )[:, tw * pair : tw * pair + tw, :],
                                in_=b_xt[
                                    :, 4 * li + tw * pair : 4 * li + tw * pair + tw, 3:515
                                ],
                            )
                    if not (tailsplit and last):
                        seng = nc.scalar if g % 2 == 0 else nc.sync
                        seng.dma_start(
                            out=y_out[i].rearrange("(t p) w -> p t w", p=128, t=4),
                            in_=b_xt[:, 4 * li : 4 * li + 4, 3:515],
                        )

    nc.finalize()
    return nc


def _get_nc():
    if "nc" not in _CACHE:
        _CACHE["nc"] = _build_nc()
    return _CACHE["nc"]


def _run_bass(x, trace=False):
    """x: (32,3,512,512) float32 -> (32,3,512,512) float32 via 8 cores."""
    from concourse.bass_utils import run_bass_kernel_spmd

    nc = _get_nc()
    xr = np.ascontiguousarray(x).reshape(N_CORES, IMGS, H, W)
    in_maps = [{"x": xr[k]} for k in range(N_CORES)]
    r = run_bass_kernel_spmd(nc, in_maps, list(range(N_CORES)), trace=trace)
    out = np.stack([r.results[k]["y"] for k in range(N_CORES)], axis=0)
    return out.reshape(32, 3, 512, 512), r


def kernel(x, se):
    x = np.asarray(x, dtype=np.float32)
    se = np.asarray(se, dtype=np.float32)
    if se.shape == (7, 7) and np.all(se == 1.0):
        out, _ = _run_bass(x)
        return out
    # general fallback (never hit for this problem's inputs)
    kh, kw = se.shape
    ph, pw = kh // 2, kw // 2
    bias = se.reshape(-1) - 1.0
    mask = (bias >= 0).astype(x.dtype)
    xp = np.pad(x, ((0, 0), (0, 0), (ph, ph), (pw, pw)))
    out = np.full(x.shape, -np.inf, dtype=x.dtype)
    for i in range(kh * kw):
        r, c = i // kw, i % kw
        win = xp[:, :, r : r + x.shape[2], c : c + x.shape[3]]
        out = np.maximum(out, mask[i] * win + bias[i])
    return out

